# revision 31
# baseline (speedup 1.0000x reference)
"""KNN block-edge kernel for Trainium2 (8 NeuronCores, one segment per core).

Problem (hardcoded from the reference):
  B=8 segments x NPER=512 blocks x U=4 units, 3-D positions, K=16.
  Candidate edges = all intra-segment block pairs (row-major, C=512 per row).
  Block-block distance = min over the 4x4 unit pairs of Euclidean distance.
  Output = per row the K nearest candidate edges, distance-ascending
  (ties: ascending edge index), as (row_o, col_o, attr) int32 arrays.

Device strategy per core (segment b):
  PE computes -d2(iu, jv) = 2*x.y - |x|^2 - |y|^2 for unit pairs.  fp32
  matmuls on the PE run as slow LOW/HIGH pairs, so operands are split into
  three bf16 terms (hi/mid/lo) with the significant cross terms stacked
  along the contract dim: one K=24 bf16 matmul per tile (error ~|v|*2^-27,
  better than the fp32 path; matmul cost only depends on streamed columns).
  Matmuls for a (row-tile t, col-tile c) pair land u-major in one 4-bank
  PSUM tile; a single VectorE tensor_reduce(max, XY) folds all 16 (u,v)
  channels at once, producing the S = -d2min chunk [128,128].  S is
  symmetric, so only the 10 upper-triangle (t<=c) chunks are computed; the
  6 mirrors are PE-transposed out of SBUF and evacuated by the (idle)
  ScalarE.  Off-diagonal chunks run before the diagonal of each row tile so
  mirror sources retire early.  During the input DMA the PE runs a warmup
  burst on zeroed operands for the HAM clock-gate.  VectorE extracts the
  per-row top-16 with max8 / max_index / match_replace (monotone in true
  distance, so sqrt is unnecessary).  Host maps local column indices
  through the actual row/col inputs; duplicate/non-monotone rows (never
  observed for this input distribution) fall back to an exact fp64 row
  recompute.
"""

import numpy as np

B = 8
NPER = 512
U = 4
KTOP = 16
NU = NPER * U          # units per segment (2048)
NBLK = B * NPER        # total blocks (4096)
MT = NPER // 128       # row tiles per core (4)
KC = 24                # contract dim of the 3-way bf16 split matmul
NEG_INF = -3.0e38
# ops column layout: [lhs t=0 | rhs c=1,2,3,0 | lhs t=1..3], each 512 cols —
# ordered by first use so the input DMA can be staged in pieces.
RHS0 = 512
LHS1 = RHS0 + NU

_cache = {}


def _lhs_col(t, u):
    return u * 128 if t == 0 else LHS1 + (t - 1) * 512 + u * 128


def _rhs_col(c):
    return RHS0 + ((c + 3) % 4) * 512


def _build_bass():
    import concourse.bacc as bacc
    import concourse.mybir as mybir
    from concourse.tile import TileContext

    bf16 = mybir.dt.bfloat16
    f32 = mybir.dt.float32
    u32 = mybir.dt.uint32

    # Bacc (not raw Bass): its compile() pass splits multi-semaphore waits —
    # TRN2 compute instructions carry at most one wait.
    nc = bacc.Bacc("TRN2")
    # Input pieces as separate contiguous DRAM tensors: a strided slice of one
    # big tensor costs ~1us per DMA dispatch (24 row descriptors); contiguous
    # pieces dispatch in one.
    ops1 = nc.dram_tensor("ops1", [KC, 1024], bf16, kind="ExternalInput")
    ops2 = nc.dram_tensor("ops2", [KC, 1024], bf16, kind="ExternalInput")
    ops3 = nc.dram_tensor("ops3", [KC, 512], bf16, kind="ExternalInput")
    ops4 = nc.dram_tensor("ops4", [KC, 1536], bf16, kind="ExternalInput")
    ident = nc.dram_tensor("ident", [128, 128], f32, kind="ExternalInput")
    # Per-tile top-16: cols [0:16] = score bits (f32), [16:32] = local index.
    out_vi = nc.dram_tensor("out_vi", [MT, 128, 2 * KTOP], u32, kind="ExternalOutput")

    with TileContext(nc) as tc:
        with (
            tc.tile_pool(name="const", bufs=1) as cpool,
            tc.tile_pool(name="psum", bufs=1, space="PSUM") as ppool,
            tc.tile_pool(name="work", bufs=2) as wpool,
            tc.tile_pool(name="topk", bufs=2) as kpool,
        ):
            # (No PE warmup burst: the PE stays at its cold 1.2 GHz clock for
            # this kernel's entire span — HAM never engages here — so warmup
            # would only delay the first real chunk.)
            wl = cpool.tile([KC, 128], bf16)
            nc.gpsimd.memset(wl, 0.0)
            warm_dummy = cpool.tile([1, 8], f32)

            ops_sb = cpool.tile([KC, U * NPER + NU], bf16)
            ident_sb = cpool.tile([128, 128], f32)
            # Staged by first use: chunk (0,1)'s operands land first, so the
            # first matmul starts as early as possible.
            nc.sync.dma_start(out=ops_sb[:, 0:1024], in_=ops1[:, :])
            nc.sync.dma_start(out=ops_sb[:, 1024:2048], in_=ops2[:, :])
            nc.sync.dma_start(out=ops_sb[:, 2048:LHS1], in_=ops3[:, :])
            nc.sync.dma_start(out=ops_sb[:, LHS1:], in_=ops4[:, :])
            nc.sync.dma_start(out=ident_sb, in_=ident[:, :])
            # Trigger the ACT Copy table load early (overlaps input DMA).
            nc.scalar.copy(out=warm_dummy, in_=wl[0:1, 0:8])

            ps_ab = [
                ppool.tile([128, NU], f32, tag="psA", name="psA"),
                ppool.tile([128, NU], f32, tag="psB", name="psB"),
            ]
            # Persistent per-tile score rows; mirrors land here via ScalarE.
            s_rows = [cpool.tile([128, NPER], f32, name=f"s{t}") for t in range(MT)]

            # Upper-triangle chunks, off-diagonal first within each row tile
            # (so mirror sources retire early); the mirror transpose of chunk
            # n is interleaved after the MM group of chunk n+1: its scratch is
            # the source chunk's own tile (parity n%2), so the only wait it
            # gains is reduce(n) — which it needs anyway for its input — and
            # chunk n+2's u=3 matmul (same tile, overlapping columns) waits
            # for the ScalarE evacuation, hidden behind its u=0..2 matmuls.
            chunks = [(t, c) for t in range(MT)
                      for c in list(range(t + 1, MT)) + [t]]
            emit_after = {m: [] for m in range(len(chunks))}
            for n, (t, c) in enumerate(chunks):
                if c > t:
                    # Phase-0/1 mirrors are deferred a few chunks further:
                    # there the DVE has top-k backlog, so the transpose's
                    # small PE stall no longer bubbles the reduce stream.
                    # Their results aren't consumed until much later top-ks.
                    delay = 4 if t == 0 else (2 if t == 1 else 1)
                    emit_after[min(n + delay, len(chunks) - 2)].append(n)

            mir = 0  # rotating scratch column slot
            for n, (t, c) in enumerate(chunks):
                s = s_rows[t]
                ps = ps_ab[n % 2]
                for u in range(U):
                    nc.tensor.matmul(
                        ps[:, u * 512:(u + 1) * 512],
                        lhsT=ops_sb[:, _lhs_col(t, u):_lhs_col(t, u) + 128],
                        rhs=ops_sb[:, _rhs_col(c):_rhs_col(c) + 512],
                        start=True,
                        stop=True,
                    )
                for nsrc in emit_after[n]:
                    ts_, cs_ = chunks[nsrc]
                    mtile = ps_ab[(n + 1) % 2]
                    sl = slice(1536 + (mir % 3) * 128, 1536 + (mir % 3 + 1) * 128)
                    mir += 1
                    nc.tensor.transpose(
                        mtile[:, sl],
                        in_=s_rows[ts_][:, cs_ * 128:(cs_ + 1) * 128],
                        identity=ident_sb,
                    )
                    nc.scalar.copy(out=s_rows[cs_][:, ts_ * 128:(ts_ + 1) * 128],
                                   in_=mtile[:, sl])
                # [128, (u j v)] -> max over u and v in one pass
                ps4 = ps.rearrange("p (u j v) -> p j u v", u=U, v=U)
                nc.vector.tensor_reduce(
                    s[:, c * 128:(c + 1) * 128], ps4,
                    mybir.AxisListType.XY, mybir.AluOpType.max,
                )
                if c != t:
                    continue
                # diagonal chunk = last for this row tile: extract its top-16.
                # Layout [v8a | i8a | v8b | i8b] so ranks 1-8 ship while
                # ranks 9-16 are still being extracted — only the second
                # half-DMA remains on the kernel's tail.
                kvi = kpool.tile([128, 2 * KTOP], u32, tag="kvi")
                s2 = wpool.tile([128, NPER], f32, tag="s2", bufs=2)
                v8a = kvi[:, 0:8].bitcast(f32)
                v8b = kvi[:, 16:24].bitcast(f32)
                nc.vector.max(out=v8a, in_=s)
                nc.vector.max_index(out=kvi[:, 8:16], in_max=v8a, in_values=s)
                nc.sync.dma_start(out=out_vi[t][:, 0:16], in_=kvi[:, 0:16])
                nc.vector.match_replace(
                    out=s2, in_to_replace=v8a, in_values=s, imm_value=NEG_INF
                )
                nc.vector.max(out=v8b, in_=s2)
                nc.vector.max_index(out=kvi[:, 24:32], in_max=v8b, in_values=s2)
                nc.sync.dma_start(out=out_vi[t][:, 16:32], in_=kvi[:, 16:32])
    nc.compile()
    return nc


def _get_nc():
    if "nc" not in _cache:
        _cache["nc"] = _build_bass()
    return _cache["nc"]


def _split3(v64):
    """Three-way bf16 split of float64 values: v ~= h + m + l."""
    import ml_dtypes
    bf = ml_dtypes.bfloat16
    h = v64.astype(bf)
    r = v64 - h.astype(np.float64)
    m = r.astype(bf)
    l = (r - m.astype(np.float64)).astype(bf)
    return h, m, l


def _make_core_inputs(unit_pos):
    """Per-core bf16 3-way-split operands.  Core b handles segment b.

    Contract rows (lhs col = block i unit u; rhs col = unit (j,v)), with
    a = 2x_lhs, y = x_rhs, n = |x|^2 (split h/m/l in float64):
      0-2  : a_h . y_h    3-5  : a_h . y_m    6-8  : a_m . y_h
      9-11 : a_m . y_m    12-14: a_h . y_l    15-17: a_l . y_h
      18-20: -n_{h,m,l} . 1          21-23: -1 . n_{h,m,l}
    so lhs.col . rhs.col = 2 x.y - n_i - n_j = -d2 exactly to ~2^-27.
    """
    import ml_dtypes
    bf = ml_dtypes.bfloat16
    ident = np.eye(128, dtype=np.float32)
    in_maps = []
    for b in range(B):
        P64 = np.ascontiguousarray(unit_pos[b * NU:(b + 1) * NU]).astype(np.float64)
        n64 = (P64 ** 2).sum(axis=1)
        ah, am, al = _split3(2.0 * P64)
        yh, ym, yl = _split3(P64)
        nh, nm, nl = _split3(n64)

        lhs = np.empty((KC, NPER * U), bf)  # u-major, reordered below
        for u in range(U):
            sl = slice(u * NPER, (u + 1) * NPER)
            lhs[0:3, sl] = ah[u::U].T
            lhs[3:6, sl] = ah[u::U].T
            lhs[6:9, sl] = am[u::U].T
            lhs[9:12, sl] = am[u::U].T
            lhs[12:15, sl] = ah[u::U].T
            lhs[15:18, sl] = al[u::U].T
            lhs[18, sl] = -nh[u::U]
            lhs[19, sl] = -nm[u::U]
            lhs[20, sl] = -nl[u::U]
            lhs[21:24, sl] = -1.0
        # reorder to [t, u, i]: col t*512 + u*128 + i  <-  u*512 + t*128 + i
        lhs_tu = lhs.reshape(KC, U, MT, 128).transpose(0, 2, 1, 3).reshape(KC, -1)

        rhs = np.empty((KC, NU), bf)
        rhs[0:3] = yh.T
        rhs[3:6] = ym.T
        rhs[6:9] = yh.T
        rhs[9:12] = ym.T
        rhs[12:15] = yl.T
        rhs[15:18] = yh.T
        rhs[18:21] = 1.0
        rhs[21] = nh
        rhs[22] = nm
        rhs[23] = nl

        ops = np.empty((KC, U * NPER + NU), bf)
        ops[:, :RHS0] = lhs_tu[:, :512]
        ops[:, LHS1:] = lhs_tu[:, 512:]
        for c in range(4):
            ops[:, _rhs_col(c):_rhs_col(c) + 512] = rhs[:, c * 512:(c + 1) * 512]
        in_maps.append({
            "ops1": np.ascontiguousarray(ops[:, 0:1024]),
            "ops2": np.ascontiguousarray(ops[:, 1024:2048]),
            "ops3": np.ascontiguousarray(ops[:, 2048:LHS1]),
            "ops4": np.ascontiguousarray(ops[:, LHS1:]),
            "ident": ident,
        })
    return in_maps


def _run_device(in_maps, trace=False):
    from concourse.bass_utils import run_bass_kernel_spmd

    nc = _get_nc()
    res = run_bass_kernel_spmd(
        nc, in_maps, core_ids=list(range(B)), trace=trace
    )
    return res


def _row_topk_f64(unit_pos, b, rloc):
    """Fallback for a repaired row: recompute its d2min in float64 and take
    the top-K with reference tie semantics (ascending d2, ties by index)."""
    P = unit_pos[b * NU:(b + 1) * NU].astype(np.float64).reshape(NPER, U, 3)
    d = P[rloc][:, None, None, :] - P[None, :, :, :]          # [U, 512, U, 3]
    d2 = np.einsum('ujvd,ujvd->ujv', d, d).min(axis=(0, 2))   # [512]
    return np.argsort(d2, kind="stable")[:KTOP]


def _postprocess(results, row, col, unit_pos):
    """Map device top-k (local j, per segment) to (row_o, col_o, attr)."""
    row_mat = row.reshape(NBLK, NPER)
    col_mat = col.reshape(NBLK, NPER)
    row_o = np.empty((NBLK, KTOP), np.int32)
    col_o = np.empty((NBLK, KTOP), np.int32)
    for b in range(B):
        vi = results[b]["out_vi"].reshape(NPER, 2 * KTOP)
        val = np.concatenate([vi[:, 0:8], vi[:, 16:24]], axis=1).view(np.float32)
        idx = np.concatenate([vi[:, 8:16], vi[:, 24:32]], axis=1).astype(np.int64)

        # Rows where a bitwise-equal score appears more than once in the
        # top-16 show duplicate indices (max_index returns the first
        # occurrence for every equal value); also guard against non-monotone
        # value order.  Neither occurs for this input distribution — if one
        # does, re-derive the row exactly on the host.
        dup = (np.sort(idx, axis=1)[:, 1:] == np.sort(idx, axis=1)[:, :-1]).any(axis=1)
        nonmono = (np.diff(val, axis=1) > 0).any(axis=1)
        for rloc in np.flatnonzero(dup | nonmono):
            idx[rloc] = _row_topk_f64(unit_pos, b, rloc)

        gr = slice(b * NPER, (b + 1) * NPER)
        rows_local = np.arange(NPER)[:, None]
        row_o[gr] = row_mat[gr][rows_local, idx]
        col_o[gr] = col_mat[gr][rows_local, idx]
    attr = np.zeros(NBLK * KTOP, np.int32)
    return row_o.reshape(-1), col_o.reshape(-1), attr


def kernel(unit_pos, row, col, unit2block, segment_ids, k):
    unit_pos = np.asarray(unit_pos, dtype=np.float32)
    row = np.asarray(row, dtype=np.int32)
    col = np.asarray(col, dtype=np.int32)
    assert int(k) == KTOP
    in_maps = _make_core_inputs(unit_pos)
    res = _run_device(in_maps, trace=False)
    return _postprocess(res.results, row, col, unit_pos)
